# revision 24
# baseline (speedup 1.0000x reference)
# Multi-head attention (B=4, T=2048, C=1024, H=16, D=64) on 8 trn2 NeuronCores.
#
# Sharding: 64 (batch, head) pairs -> 8 per core. Core c handles batch c//2,
# heads 8*(c%2) .. 8*(c%2)+8, i.e. a contiguous [2048, 512] column slice of x
# (and of the output). Q/K/V weights are tiny and replicated (pre-processed on
# host into block-diagonal lhsT form so two heads share one 128-contraction).
#
# Per-core pipeline, fully software-pipelined:
#   Phase A (upfront, all 4 head-pairs): load x slice, PE-transpose it,
#     project QT2/KT2 (f32r, bias via ScalarE) and V2A/V2B (bf16, ones col 64
#     to accumulate softmax denominators for free). Everything stays in SBUF.
#   Phase B: flat 256-step flash loop over (pair, chunk, si) with 2-step
#     lookahead: at step t issue S.T = K.T@Q (PSUM halves), exp (ScalarE for
#     10/16 si, Schraudolph int16 trick on VectorE for 6/16 - same split and
#     therefore same numerics as the reference-passing baseline), and the PV
#     accumulation for step t-2. The PE therefore never waits on exp.
#   Out-stage per (pair, chunk): O.T copied PSUM->SBUF (stA on VectorE, stB
#     on ScalarE so they drain in parallel), transposed back via PE in [65,128]
#     chunks two steps later, r = 1/denominator, out = O*r + bv, DMA out.
import numpy as np

B, T, C = 4, 2048, 1024
H, D = 16, 64
NCORES = 8
PCOLS = C // 2          # 512 columns per core
TO = T // 128           # 16 t tiles
NPAIR = PCOLS // 128    # 4 head pairs per core

_cached_nc = None

SCHR_SET = ()   # all-ScalarE exp: Act exp measured 556ns/tile on HW


def _build_nc(reps=1, variant="full"):
    import concourse.bass as bass
    import concourse.mybir as mybir
    import concourse.tile as tile
    from concourse import bacc
    from concourse.masks import make_identity

    f32 = mybir.dt.float32
    f32r = mybir.dt.float32r
    bf16 = mybir.dt.bfloat16
    i16 = mybir.dt.int16
    f16 = mybir.dt.float16
    f8e4 = mybir.dt.float8e4
    MPM = mybir.MatmulPerfMode
    AF = mybir.ActivationFunctionType
    ALU = mybir.AluOpType

    SCHRA_A = float(np.float32((1 << 7) / np.log(2.0) * 0.125))
    SCHRA_B = float(np.float32(127.0 * 128 - 5.0))
    nc = bacc.Bacc("TRN2", target_bir_lowering=False, debug=False)

    xs = nc.dram_tensor("xs", [T, PCOLS], f32, kind="ExternalInput")
    wq2 = nc.dram_tensor("wq2", [128, 128], f32, kind="ExternalInput")
    wk2 = nc.dram_tensor("wk2", [128, 128], f32, kind="ExternalInput")
    wv2 = nc.dram_tensor("wv2", [128, 256], f32, kind="ExternalInput")
    bq2 = nc.dram_tensor("bq2", [128, 1], f32, kind="ExternalInput")
    bk2 = nc.dram_tensor("bk2", [128, 1], f32, kind="ExternalInput")
    bvb = nc.dram_tensor("bvb", [128, 64], f32, kind="ExternalInput")
    ys = nc.dram_tensor("ys", [T, PCOLS], f32, kind="ExternalOutput")

    x_r = xs[:].rearrange("(to p) c -> p to c", p=128)   # [128, 16, 512]
    y_r = ys[:].rearrange("(to p) c -> p to c", p=128)   # [128, 16, 512]

    with tile.TileContext(nc) as tc:
        from contextlib import ExitStack

        with ExitStack() as ctx:
            const = ctx.enter_context(tc.tile_pool(name="const", bufs=1))
            bigp = ctx.enter_context(tc.tile_pool(name="bigp", bufs=1))
            ptp = ctx.enter_context(tc.tile_pool(name="ptp", bufs=4))
            stp = ctx.enter_context(tc.tile_pool(name="stp", bufs=2))
            otp = ctx.enter_context(tc.tile_pool(name="otp", bufs=2))
            osp = ctx.enter_context(tc.tile_pool(name="osp", bufs=2))
            smallp = ctx.enter_context(tc.tile_pool(name="smallp", bufs=4))
            # PSUM: 3 x [128,1024] f32 score tiles / transposes /
            # projections (2 banks each) + oA (1 bank) + oB (1 bank) = 8.
            ps_s = ctx.enter_context(tc.tile_pool(name="ps_s", bufs=3, space="PSUM"))
            ps_o = ctx.enter_context(tc.tile_pool(name="ps_o", bufs=1, space="PSUM"))

            ident = const.tile([128, 128], f32)
            make_identity(nc, ident)
            # Dummy PE transpose so the PE observes gpsimd's identity write
            # here; otherwise the first real transpose needs two semaphore
            # waits (gpsimd + DMA) and walrus allows only one on the
            # transpose-mode LDWEIGHTS struct.
            pst0 = ps_s.tile([128, 128], f32, tag="s")
            nc.tensor.transpose(pst0, ident, ident)
            wq2_sb = const.tile([128, 128], f32)
            wk2_sb = const.tile([128, 128], f32)
            wv2_sb = const.tile([128, 256], f32)
            wq2_r = const.tile([128, 128], bf16)
            wk2_r = const.tile([128, 128], bf16)
            wv2_r = const.tile([128, 256], bf16)
            bq2_sb = const.tile([128, 1], f32)
            bk2_sb = const.tile([128, 1], f32)
            bvb_sb = const.tile([128, 64], f32)
            bvb16 = const.tile([128, 64], f16)
            ones16 = const.tile([128, TO, 1], f32)
            nc.vector.memset(ones16[:], 1.0)
            ptc = const.tile([128, 1024], bf16)
            nc.vector.memset(ptc[:], 0.001)
            nc.sync.dma_start(wq2_sb[:], wq2[:])
            nc.sync.dma_start(wk2_sb[:], wk2[:])
            nc.sync.dma_start(wv2_sb[:], wv2[:])
            nc.sync.dma_start(bq2_sb[:], bq2[:])
            nc.sync.dma_start(bk2_sb[:], bk2[:])
            nc.sync.dma_start(bvb_sb[:], bvb[:])
            nc.vector.tensor_copy(bvb16[:], bvb_sb[:])
            nc.vector.tensor_copy(wq2_r[:], wq2_sb[:])
            nc.vector.tensor_copy(wk2_r[:], wk2_sb[:])
            nc.vector.tensor_copy(wv2_r[:], wv2_sb[:])

            import contextlib
            loop_cm = tc.For_i(0, reps, 1) if reps > 1 else \
                contextlib.nullcontext()
            with loop_cm:
                # ---- Phase A: load x, transpose, project all 4 pairs ----
                xsb = bigp.tile([128, TO, PCOLS], f32, tag="xsb")
                for g in range(8):
                    eng = nc.sync if g % 2 == 0 else nc.scalar
                    eng.dma_start(xsb[:, 2 * g:2 * g + 2, :],
                                  x_r[:, 2 * g:2 * g + 2, :])

                QT = [None] * NPAIR
                KT = [None] * NPAIR
                VV = [None] * NPAIR
                for p in range(NPAIR):
                    xT2 = bigp.tile([128, TO, 128], bf16, tag=f"xT{p}")
                    for to in range(TO):
                        pst = ps_s.tile([128, 128], f32, tag="s")
                        nc.tensor.transpose(
                            pst, xsb[:, to, p * 128:(p + 1) * 128], ident)
                        nc.vector.tensor_copy(xT2[:, to, :], pst)

                    QT2 = bigp.tile([128, TO, 128], bf16, tag=f"qt{p}")
                    KT2 = bigp.tile([128, TO, 128], bf16, tag=f"kt{p}")
                    for chk in range(4):
                        rhs = xT2[:, 4 * chk:4 * chk + 4, :]
                        psq = ps_s.tile([128, 512], f32, tag="s")
                        nc.tensor.matmul(psq, wq2_r[:], rhs,
                                         start=True, stop=True)
                        nc.scalar.activation(
                            QT2[:, 4 * chk:4 * chk + 4, :], psq,
                            AF.Identity, bias=bq2_sb[:])
                        psk = ps_s.tile([128, 512], f32, tag="s")
                        nc.tensor.matmul(psk, wk2_r[:], rhs,
                                         start=True, stop=True)
                        # K bias dropped: (q+bq)@(k+bk) differs from
                        # (q+bq)@k only by per-query constants, which
                        # cancel in softmax. Plain copy, split Act/DVE.
                        ksl = KT2[:, 4 * chk:4 * chk + 4, :]
                        if chk % 2 == 0:
                            nc.scalar.activation(ksl, psk, AF.Identity)
                        else:
                            nc.vector.tensor_copy(ksl, psk)

                    # V2: heads A and B in one tile; ones at cols 64, 129
                    # accumulate the softmax denominators for free.
                    V2 = bigp.tile([128, TO, 130], bf16, tag=f"v{p}")
                    nc.vector.tensor_copy(V2[:, :, 64:65], ones16[:])
                    nc.vector.tensor_copy(V2[:, :, 129:130], ones16[:])
                    v2v = V2[:].rearrange("q to (h e) -> q to h e", e=65)
                    for to in range(TO):
                        psv = ps_s.tile([128, 128], f32, tag="s")
                        nc.tensor.matmul(psv, xT2[:, to, :],
                                         wv2_r[:, 0:128],
                                         start=True, stop=True)
                        if to % 2 == 0:
                            nc.scalar.activation(
                                v2v[:, to, :, 0:64],
                                psv.rearrange("q (h e) -> q h e", e=64),
                                AF.Identity)
                        else:
                            nc.vector.tensor_copy(
                                v2v[:, to, :, 0:64],
                                psv.rearrange("q (h e) -> q h e", e=64))
                    QT[p], KT[p], VV[p] = QT2, KT2, V2

                # ---- Phase B: pipelined flash loop over 256 (p,ch,si) ----
                NSTEP = NPAIR * 4 * TO    # 256
                LOOK = 2
                rhs_store = {}
                pair_tmp = {}
                o_cur = {}
                pending = {}

                def qk_exp(h):
                    p, ch, si = h // 64, (h // 16) % 4, h % 16
                    QT2, KT2 = QT[p], KT[p]
                    qs = QT2[:, 4 * ch:4 * ch + 4, :]
                    sAB = ps_s.tile([128, 1024], f32, tag="s")
                    nc.tensor.matmul(sAB[:, 0:512], KT2[0:64, si, :],
                                     qs[0:64], start=True, stop=True)
                    nc.tensor.matmul(sAB[:, 512:1024], KT2[64:128, si, :],
                                     qs[64:128], start=True, stop=True)
                    if variant in ("qk", "qkpv"):
                        rhs_store[h] = (ptc[:, 0:512], ptc[:, 512:1024])
                        return
                    ptAB = ptp.tile([128, 1024], bf16, tag="pt")
                    nc.scalar.activation(ptAB, sAB, AF.Exp, scale=0.125)
                    rhs_store[h] = (ptAB[:, 0:512], ptAB[:, 512:1024])

                def out_finish(p, ch, stA, stB):
                    # O.T (fp16) -> O via the DMA xbar transpose engine:
                    # 16x128 tiles, no PE involvement at all.
                    ostT = otp.tile([128, 2, 4, 80], f16, tag="ostT")
                    for k in range(4):
                        nc.sync.dma_start_transpose(
                            ostT[:, 0, k, :], stA[:, k * 128:(k + 1) * 128])
                        nc.sync.dma_start_transpose(
                            ostT[:, 1, k, :], stB[:, k * 128:(k + 1) * 128])
                    ost = osp.tile([128, 4, 128], f32, tag="ost")
                    rr = smallp.tile([128, 2, 4, 1], f32, tag="r")
                    nc.vector.reciprocal(rr, ostT[:, :, :, 64:65])
                    for k in range(4):
                        nc.vector.scalar_tensor_tensor(
                            out=ost[:, k, 0:64], in0=ostT[:, 0, k, 0:64],
                            scalar=rr[:, 0, k, :], in1=bvb16[:],
                            op0=ALU.mult, op1=ALU.add)
                        nc.vector.scalar_tensor_tensor(
                            out=ost[:, k, 64:128], in0=ostT[:, 1, k, 0:64],
                            scalar=rr[:, 1, k, :], in1=bvb16[:],
                            op0=ALU.mult, op1=ALU.add)
                    nc.sync.dma_start(
                        y_r[:, 4 * ch:4 * ch + 4, p * 128:(p + 1) * 128],
                        ost[:])

                def pv(h, t):
                    p, ch, si = h // 64, (h // 16) % 4, h % 16
                    rhsA, rhsB = rhs_store.pop(h)
                    if variant in ("qk", "qkexp"):
                        return
                    if si == 0:
                        o_cur["A"] = ps_o.tile([65, 512], f32, tag="oA", name="oA")
                        o_cur["B"] = ps_o.tile([65, 512], f32, tag="oB", name="oB")
                    oA_t, oB_t = o_cur["A"], o_cur["B"]
                    nc.tensor.matmul(oA_t, VV[p][:, si, 0:65], rhsA,
                                     start=(si == 0), stop=(si == TO - 1))
                    nc.tensor.matmul(oB_t, VV[p][:, si, 65:130], rhsB,
                                     start=(si == 0), stop=(si == TO - 1))
                    if si == TO - 1:
                        stA = stp.tile([80, 512], f16, tag="stA")
                        nc.gpsimd.memset(stA[64:80, :], 0)
                        nc.vector.tensor_scalar(
                            out=stA[0:65, :], in0=oA_t,
                            scalar1=0.0625, scalar2=None, op0=ALU.mult)
                        stB = stp.tile([80, 512], f16, tag="stB")
                        nc.gpsimd.memset(stB[64:80, :], 0)
                        nc.vector.tensor_scalar(
                            out=stB[0:65, :], in0=oB_t,
                            scalar1=0.0625, scalar2=None, op0=ALU.mult)
                        pending.setdefault(t + 1, []).append(
                            (p, ch, stA, stB))

                for t in range(NSTEP + LOOK + 3):
                    if t < NSTEP:
                        qk_exp(t)
                    for args in pending.pop(t, ()):
                        out_finish(*args)
                    if LOOK <= t < NSTEP + LOOK:
                        pv(t - LOOK, t)
    nc.compile()
    return nc


def _host_inputs(x, Wq, bq, Wk, bk, Wv, bv):
    def blockdiag(w):
        out = np.zeros((128, 128), dtype=np.float32)
        out[0:64, 0:64] = w
        out[64:128, 64:128] = w
        return out

    wq2 = blockdiag(np.ascontiguousarray(Wq.T))
    wk2 = blockdiag(np.ascontiguousarray(Wk.T))
    wv2_1 = blockdiag(np.ascontiguousarray(Wv.T))
    wv2 = np.ascontiguousarray(np.concatenate([wv2_1, wv2_1], axis=1))
    bq2 = np.concatenate([bq, bq]).reshape(128, 1).astype(np.float32)
    bk2 = np.concatenate([bk, bk]).reshape(128, 1).astype(np.float32)
    bvb = np.tile(bv.reshape(1, 64), (128, 1)).astype(np.float32)

    in_maps = []
    for c in range(NCORES):
        b, half = c // 2, c % 2
        xsl = np.ascontiguousarray(x[b, :, half * PCOLS:(half + 1) * PCOLS],
                                   dtype=np.float32)
        in_maps.append({
            "xs": xsl, "wq2": wq2, "wk2": wk2, "wv2": wv2,
            "bq2": bq2, "bk2": bk2, "bvb": bvb,
        })
    return in_maps


def _run(x, Wq, bq, Wk, bk, Wv, bv, trace=False):
    from concourse.bass_utils import run_bass_kernel_spmd

    global _cached_nc
    if _cached_nc is None:
        _cached_nc = _build_nc()
    in_maps = _host_inputs(x, Wq, bq, Wk, bk, Wv, bv)
    res = run_bass_kernel_spmd(_cached_nc, in_maps,
                               core_ids=list(range(NCORES)), trace=trace)
    y = np.empty((B, T, C), dtype=np.float32)
    for c in range(NCORES):
        b, half = c // 2, c % 2
        y[b, :, half * PCOLS:(half + 1) * PCOLS] = res.results[c]["ys"]
    return y, res


def kernel(x, Wq, bq, Wk, bk, Wv, bv):
    y, _ = _run(np.asarray(x), np.asarray(Wq), np.asarray(bq), np.asarray(Wk),
                np.asarray(bk), np.asarray(Wv), np.asarray(bv))
    return y


# revision 25
# speedup vs baseline: 1.0026x; 1.0026x over previous
# Multi-head attention (B=4, T=2048, C=1024, H=16, D=64) on 8 trn2 NeuronCores.
#
# Sharding: 64 (batch, head) pairs -> 8 per core. Core c handles batch c//2,
# heads 8*(c%2) .. 8*(c%2)+8, i.e. a contiguous [2048, 512] column slice of x
# (and of the output). Q/K/V weights are tiny and replicated (pre-processed on
# host into block-diagonal lhsT form so two heads share one 128-contraction).
#
# Per-core pipeline (HW-tuned: the kernel is matmul-ISSUE-bound at ~300ns per
# 512-col matmul on real silicon, so everything else hides behind the PE):
#   Phase A (upfront, all 4 head-pairs, all tensors stay in SBUF):
#     load x, PE-transpose it, project QT2/KT2/V2 in bf16 (bf16 Q/K verified
#     numerically equivalent to fp32r; fp32r streams 2x slower on HW).
#     K bias dropped: it only adds per-query constants to scores, which
#     cancel in softmax. V gets "ones" columns so PV accumulates the softmax
#     denominators for free.
#   Phase B: flat 256-step flash loop over (pair, chunk, si), software-
#     pipelined with 2-step lookahead: step t issues S.T = K.T@Q (PSUM
#     [128,1024]), exp on ScalarE only (556ns/tile measured; the Schraudolph
#     DVE trick is slower on HW and was the chain bottleneck), and the PV
#     accumulation for step t-2. PSUM: 3x scores (6 banks) + oA + oB = 8.
#   Out-stage per (pair, chunk), PE-free: O.T scaled 1/16 into fp16
#     [80,512] tiles (pad rows zeroed on the idle GpSimd engine), transposed
#     by the DMA xbar engine (16x128 tiles, SP queue only - the Act-queue
#     xbar path races), then r = 1/denominator and out = O*r + bv on DVE.
import numpy as np

B, T, C = 4, 2048, 1024
H, D = 16, 64
NCORES = 8
PCOLS = C // 2          # 512 columns per core
TO = T // 128           # 16 t tiles
NPAIR = PCOLS // 128    # 4 head pairs per core

_cached_nc = None

SCHR_SET = ()   # all-ScalarE exp: Act exp measured 556ns/tile on HW


def _build_nc(reps=1, variant="full"):
    import concourse.bass as bass
    import concourse.mybir as mybir
    import concourse.tile as tile
    from concourse import bacc
    from concourse.masks import make_identity

    f32 = mybir.dt.float32
    f32r = mybir.dt.float32r
    bf16 = mybir.dt.bfloat16
    i16 = mybir.dt.int16
    f16 = mybir.dt.float16
    f8e4 = mybir.dt.float8e4
    MPM = mybir.MatmulPerfMode
    AF = mybir.ActivationFunctionType
    ALU = mybir.AluOpType

    SCHRA_A = float(np.float32((1 << 7) / np.log(2.0) * 0.125))
    SCHRA_B = float(np.float32(127.0 * 128 - 5.0))
    nc = bacc.Bacc("TRN2", target_bir_lowering=False, debug=False)

    xs = nc.dram_tensor("xs", [T, PCOLS], f32, kind="ExternalInput")
    wq2 = nc.dram_tensor("wq2", [128, 128], f32, kind="ExternalInput")
    wk2 = nc.dram_tensor("wk2", [128, 128], f32, kind="ExternalInput")
    wv2 = nc.dram_tensor("wv2", [128, 256], f32, kind="ExternalInput")
    bq2 = nc.dram_tensor("bq2", [128, 1], f32, kind="ExternalInput")
    bk2 = nc.dram_tensor("bk2", [128, 1], f32, kind="ExternalInput")
    bvb = nc.dram_tensor("bvb", [128, 64], f32, kind="ExternalInput")
    ys = nc.dram_tensor("ys", [T, PCOLS], f32, kind="ExternalOutput")

    x_r = xs[:].rearrange("(to p) c -> p to c", p=128)   # [128, 16, 512]
    y_r = ys[:].rearrange("(to p) c -> p to c", p=128)   # [128, 16, 512]

    with tile.TileContext(nc) as tc:
        from contextlib import ExitStack

        with ExitStack() as ctx:
            const = ctx.enter_context(tc.tile_pool(name="const", bufs=1))
            bigp = ctx.enter_context(tc.tile_pool(name="bigp", bufs=1))
            ptp = ctx.enter_context(tc.tile_pool(name="ptp", bufs=4))
            stp = ctx.enter_context(tc.tile_pool(name="stp", bufs=2))
            otp = ctx.enter_context(tc.tile_pool(name="otp", bufs=2))
            osp = ctx.enter_context(tc.tile_pool(name="osp", bufs=2))
            smallp = ctx.enter_context(tc.tile_pool(name="smallp", bufs=4))
            # PSUM: 3 x [128,1024] f32 score tiles / transposes /
            # projections (2 banks each) + oA (1 bank) + oB (1 bank) = 8.
            ps_s = ctx.enter_context(tc.tile_pool(name="ps_s", bufs=3, space="PSUM"))
            ps_o = ctx.enter_context(tc.tile_pool(name="ps_o", bufs=1, space="PSUM"))

            ident = const.tile([128, 128], f32)
            make_identity(nc, ident)
            # Dummy PE transpose so the PE observes gpsimd's identity write
            # here; otherwise the first real transpose needs two semaphore
            # waits (gpsimd + DMA) and walrus allows only one on the
            # transpose-mode LDWEIGHTS struct.
            pst0 = ps_s.tile([128, 128], f32, tag="s")
            nc.tensor.transpose(pst0, ident, ident)
            wq2_sb = const.tile([128, 128], f32)
            wk2_sb = const.tile([128, 128], f32)
            wv2_sb = const.tile([128, 256], f32)
            wq2_r = const.tile([128, 128], bf16)
            wk2_r = const.tile([128, 128], bf16)
            wv2_r = const.tile([128, 256], bf16)
            bq2_sb = const.tile([128, 1], f32)
            bk2_sb = const.tile([128, 1], f32)
            bvb_sb = const.tile([128, 64], f32)
            bvb16 = const.tile([128, 64], f16)
            ones16 = const.tile([128, TO, 1], f32)
            nc.vector.memset(ones16[:], 1.0)
            ptc = const.tile([128, 1024], bf16)
            nc.vector.memset(ptc[:], 0.001)
            nc.sync.dma_start(wq2_sb[:], wq2[:])
            nc.sync.dma_start(wk2_sb[:], wk2[:])
            nc.sync.dma_start(wv2_sb[:], wv2[:])
            nc.sync.dma_start(bq2_sb[:], bq2[:])
            nc.sync.dma_start(bk2_sb[:], bk2[:])
            nc.sync.dma_start(bvb_sb[:], bvb[:])
            nc.vector.tensor_copy(bvb16[:], bvb_sb[:])
            nc.vector.tensor_copy(wq2_r[:], wq2_sb[:])
            nc.vector.tensor_copy(wk2_r[:], wk2_sb[:])
            nc.vector.tensor_copy(wv2_r[:], wv2_sb[:])

            import contextlib
            loop_cm = tc.For_i(0, reps, 1) if reps > 1 else \
                contextlib.nullcontext()
            with loop_cm:
                # ---- Phase A: load x, transpose, project all 4 pairs ----
                xsb = bigp.tile([128, TO, PCOLS], f32, tag="xsb")
                for g in range(8):
                    eng = nc.sync if g % 2 == 0 else nc.scalar
                    eng.dma_start(xsb[:, 2 * g:2 * g + 2, :],
                                  x_r[:, 2 * g:2 * g + 2, :])

                QT = [None] * NPAIR
                KT = [None] * NPAIR
                VV = [None] * NPAIR
                for p in range(NPAIR):
                    xT2 = bigp.tile([128, TO, 128], bf16, tag=f"xT{p}")
                    for to in range(TO):
                        pst = ps_s.tile([128, 128], f32, tag="s")
                        nc.tensor.transpose(
                            pst, xsb[:, to, p * 128:(p + 1) * 128], ident)
                        nc.vector.tensor_copy(xT2[:, to, :], pst)

                    QT2 = bigp.tile([128, TO, 128], bf16, tag=f"qt{p}")
                    KT2 = bigp.tile([128, TO, 128], bf16, tag=f"kt{p}")
                    for chk in range(4):
                        rhs = xT2[:, 4 * chk:4 * chk + 4, :]
                        psq = ps_s.tile([128, 512], f32, tag="s")
                        nc.tensor.matmul(psq, wq2_r[:], rhs,
                                         start=True, stop=True)
                        nc.scalar.activation(
                            QT2[:, 4 * chk:4 * chk + 4, :], psq,
                            AF.Identity, bias=bq2_sb[:])
                        psk = ps_s.tile([128, 512], f32, tag="s")
                        nc.tensor.matmul(psk, wk2_r[:], rhs,
                                         start=True, stop=True)
                        # K bias dropped: (q+bq)@(k+bk) differs from
                        # (q+bq)@k only by per-query constants, which
                        # cancel in softmax. Plain copy, split Act/DVE.
                        ksl = KT2[:, 4 * chk:4 * chk + 4, :]
                        if chk % 2 == 0:
                            nc.scalar.activation(ksl, psk, AF.Identity)
                        else:
                            nc.vector.tensor_copy(ksl, psk)

                    # V2: heads A and B in one tile; ones at cols 64, 129
                    # accumulate the softmax denominators for free.
                    V2 = bigp.tile([128, TO, 130], bf16, tag=f"v{p}")
                    nc.vector.tensor_copy(V2[:, :, 64:65], ones16[:])
                    nc.vector.tensor_copy(V2[:, :, 129:130], ones16[:])
                    v2v = V2[:].rearrange("q to (h e) -> q to h e", e=65)
                    for to in range(TO):
                        psv = ps_s.tile([128, 128], f32, tag="s")
                        nc.tensor.matmul(psv, xT2[:, to, :],
                                         wv2_r[:, 0:128],
                                         start=True, stop=True)
                        if to % 2 == 0:
                            nc.scalar.activation(
                                v2v[:, to, :, 0:64],
                                psv.rearrange("q (h e) -> q h e", e=64),
                                AF.Identity)
                        else:
                            nc.vector.tensor_copy(
                                v2v[:, to, :, 0:64],
                                psv.rearrange("q (h e) -> q h e", e=64))
                    QT[p], KT[p], VV[p] = QT2, KT2, V2

                # ---- Phase B: pipelined flash loop over 256 (p,ch,si) ----
                NSTEP = NPAIR * 4 * TO    # 256
                LOOK = 2
                rhs_store = {}
                pair_tmp = {}
                o_cur = {}
                pending = {}

                def qk_exp(h):
                    p, ch, si = h // 64, (h // 16) % 4, h % 16
                    QT2, KT2 = QT[p], KT[p]
                    qs = QT2[:, 4 * ch:4 * ch + 4, :]
                    sAB = ps_s.tile([128, 1024], f32, tag="s")
                    nc.tensor.matmul(sAB[:, 0:512], KT2[0:64, si, :],
                                     qs[0:64], start=True, stop=True)
                    nc.tensor.matmul(sAB[:, 512:1024], KT2[64:128, si, :],
                                     qs[64:128], start=True, stop=True)
                    if variant in ("qk", "qkpv"):
                        rhs_store[h] = (ptc[:, 0:512], ptc[:, 512:1024])
                        return
                    ptAB = ptp.tile([128, 1024], bf16, tag="pt")
                    nc.scalar.activation(ptAB, sAB, AF.Exp, scale=0.125)
                    rhs_store[h] = (ptAB[:, 0:512], ptAB[:, 512:1024])

                def out_finish(p, ch, stA, stB):
                    # O.T (fp16) -> O via the DMA xbar transpose engine:
                    # 16x128 tiles, no PE involvement at all.
                    ostT = otp.tile([128, 2, 4, 80], f16, tag="ostT")
                    for k in range(4):
                        nc.sync.dma_start_transpose(
                            ostT[:, 0, k, :], stA[:, k * 128:(k + 1) * 128])
                        nc.sync.dma_start_transpose(
                            ostT[:, 1, k, :], stB[:, k * 128:(k + 1) * 128])
                    ost = osp.tile([128, 4, 128], f32, tag="ost")
                    rr = smallp.tile([128, 2, 4, 1], f32, tag="r")
                    nc.vector.reciprocal(rr, ostT[:, :, :, 64:65])
                    for k in range(4):
                        nc.vector.scalar_tensor_tensor(
                            out=ost[:, k, 0:64], in0=ostT[:, 0, k, 0:64],
                            scalar=rr[:, 0, k, :], in1=bvb16[:],
                            op0=ALU.mult, op1=ALU.add)
                        nc.vector.scalar_tensor_tensor(
                            out=ost[:, k, 64:128], in0=ostT[:, 1, k, 0:64],
                            scalar=rr[:, 1, k, :], in1=bvb16[:],
                            op0=ALU.mult, op1=ALU.add)
                    nc.sync.dma_start(
                        y_r[:, 4 * ch:4 * ch + 4, p * 128:(p + 1) * 128],
                        ost[:])

                def pv(h, t):
                    p, ch, si = h // 64, (h // 16) % 4, h % 16
                    rhsA, rhsB = rhs_store.pop(h)
                    if variant in ("qk", "qkexp"):
                        return
                    if si == 0:
                        o_cur["A"] = ps_o.tile([65, 512], f32, tag="oA", name="oA")
                        o_cur["B"] = ps_o.tile([65, 512], f32, tag="oB", name="oB")
                    oA_t, oB_t = o_cur["A"], o_cur["B"]
                    nc.tensor.matmul(oA_t, VV[p][:, si, 0:65], rhsA,
                                     start=(si == 0), stop=(si == TO - 1))
                    nc.tensor.matmul(oB_t, VV[p][:, si, 65:130], rhsB,
                                     start=(si == 0), stop=(si == TO - 1))
                    if si == TO - 1:
                        stA = stp.tile([80, 512], f16, tag="stA")
                        nc.gpsimd.memset(stA[64:80, :], 0)
                        nc.vector.tensor_scalar(
                            out=stA[0:65, :], in0=oA_t,
                            scalar1=0.0625, scalar2=None, op0=ALU.mult)
                        stB = stp.tile([80, 512], f16, tag="stB")
                        nc.gpsimd.memset(stB[64:80, :], 0)
                        nc.vector.tensor_scalar(
                            out=stB[0:65, :], in0=oB_t,
                            scalar1=0.0625, scalar2=None, op0=ALU.mult)
                        pending.setdefault(t + 1, []).append(
                            (p, ch, stA, stB))

                for t in range(NSTEP + LOOK + 3):
                    if t < NSTEP:
                        qk_exp(t)
                    for args in pending.pop(t, ()):
                        out_finish(*args)
                    if LOOK <= t < NSTEP + LOOK:
                        pv(t - LOOK, t)
    nc.compile()
    return nc


def _host_inputs(x, Wq, bq, Wk, bk, Wv, bv):
    def blockdiag(w):
        out = np.zeros((128, 128), dtype=np.float32)
        out[0:64, 0:64] = w
        out[64:128, 64:128] = w
        return out

    wq2 = blockdiag(np.ascontiguousarray(Wq.T))
    wk2 = blockdiag(np.ascontiguousarray(Wk.T))
    wv2_1 = blockdiag(np.ascontiguousarray(Wv.T))
    wv2 = np.ascontiguousarray(np.concatenate([wv2_1, wv2_1], axis=1))
    bq2 = np.concatenate([bq, bq]).reshape(128, 1).astype(np.float32)
    bk2 = np.concatenate([bk, bk]).reshape(128, 1).astype(np.float32)
    bvb = np.tile(bv.reshape(1, 64), (128, 1)).astype(np.float32)

    in_maps = []
    for c in range(NCORES):
        b, half = c // 2, c % 2
        xsl = np.ascontiguousarray(x[b, :, half * PCOLS:(half + 1) * PCOLS],
                                   dtype=np.float32)
        in_maps.append({
            "xs": xsl, "wq2": wq2, "wk2": wk2, "wv2": wv2,
            "bq2": bq2, "bk2": bk2, "bvb": bvb,
        })
    return in_maps


def _run(x, Wq, bq, Wk, bk, Wv, bv, trace=False):
    from concourse.bass_utils import run_bass_kernel_spmd

    global _cached_nc
    if _cached_nc is None:
        _cached_nc = _build_nc()
    in_maps = _host_inputs(x, Wq, bq, Wk, bk, Wv, bv)
    res = run_bass_kernel_spmd(_cached_nc, in_maps,
                               core_ids=list(range(NCORES)), trace=trace)
    y = np.empty((B, T, C), dtype=np.float32)
    for c in range(NCORES):
        b, half = c // 2, c % 2
        y[b, :, half * PCOLS:(half + 1) * PCOLS] = res.results[c]["ys"]
    return y, res


def kernel(x, Wq, bq, Wk, bk, Wv, bv):
    y, _ = _run(np.asarray(x), np.asarray(Wq), np.asarray(bq), np.asarray(Wk),
                np.asarray(bk), np.asarray(Wv), np.asarray(bv))
    return y
